# revision 23
# baseline (speedup 1.0000x reference)
"""Trainium2 Bass/Tile kernel for EntropyRecyclingLanguageNet (vq_codebook).

Computes, for x[B,D]:
    pw    = softmax(x @ attn_w + attn_b)               # [B,P]
    rec   = pw @ pattern_dict                          # [B,D]
    par   = rec @ self_w + self_b - rec                # [B,D]
    out   = (rec * sigmoid(||par||)) @ out_w + out_b   # [B,V]

Sharding: tensor-parallel over the vocab dim (V=32000 -> 4000 per core);
every core runs the full small stage for all B rows (cheap), and the
dominant cost -- writing the [8192, 4000] output slice -- is spread
across the 8 cores.  Host gathers with a concat along axis 1.

The kernel is memory-bound on the output write, so the whole heavy data
path runs in fp16 (tolerance is 2e-2; measured fp16 rel err ~4e-4):
fp16 output halves HBM write traffic vs f32.

Structure (per core):
  * weight folds on host (input-independent):
      m2   = pattern_dict @ out_w                  [P, V]   (vocab-sharded)
      m3x  = [[pattern_dict @ (self_w - I) | 1(P)];
              [self_b                      | 0  ]] [P+1, D+1]
    out_b is added on the host during the gather (it is typically zero).
  * phase A per 512-wide block: logitsT = attn_w.T @ xT-block on PE;
    ACT Exp with attn_b bias writes unnormalized expwT rows of a
    persistent ewT buffer [P+1, B]; a ones-row matmul fills row P with
    the softmax denominators.
  * phase A2 per 128-col tile: ONE K=65 matmul ewT_tile.T @ m3x gives
    parScaled = denom*par in cols 0:D and the denominator column in
    col D.  ACT Square with accum_out produces q = ||parScaled||^2.
  * gate per group of 8 tiles, using only Ln/Exp (+ DVE recip/mul) so
    the ACT table set never switches:
      scl = sigmoid(sqrt(q)/denom) / denom,  sqrt(q) = exp(0.5 ln q),
      sigmoid(m) = 1/(1 + exp(-m))
  * phase B per tile: out_tile = ewT_tile.T @ m2 on PE (K=64, 512-wide
    PSUM chunks); the per-row scale scl is applied during the
    PSUM->SBUF drain (ACT activation-with-scale / DVE tensor_scalar),
    split across both engines; fp16 SBUF tile DMAs straight out.
"""

import numpy as np

import concourse.bass as bass
import concourse.mybir as mybir
import concourse.tile as tile
from concourse import bacc
from concourse.bass_utils import run_bass_kernel_spmd

# The greedy act-table pass picks the first set containing each function:
# Exp -> exp_and_others, Ln -> natural_log, which thrash (~17 reloads,
# 2.7us each).  Every ACT function this kernel uses (exp, ln, square,
# identity, copy, memset_zero) lives in natural_log_exp_and_others, so
# blank out every other set: the pass then emits exactly one table load.
# Set ids stay aligned with act_info.json (only values are emptied).
_KEEP_SET = "natural_log_exp_and_others"
_orig_gat = bacc.get_activation_tables


def _single_set_tables(arch):
    return {k: (v if k == _KEEP_SET else set()) for k, v in _orig_gat(arch).items()}


if not __import__("os").environ.get("NO_ACT_PATCH"):
    bacc.get_activation_tables = _single_set_tables

B, D, P, V = 8192, 128, 64, 32000
NCORES = 8
VS = V // NCORES        # vocab cols per core (4000)
BT = 128                # batch tile (partition dim)
NBT = B // BT           # 64 batch tiles
G = 8                   # batch tiles per gate group
W = 512                 # phase-A block width (4 batch tiles)
PCH = 1024              # PSUM proj tile width (2 banks; 2 matmuls each)
ACT_FRAC = 0.57         # fraction of projection drains on the scalar engine
F16 = mybir.dt.float16
F32 = mybir.dt.float32
AF = mybir.ActivationFunctionType

_cache = {}


def _build():
    nc = bacc.Bacc(
        "TRN2",
        target_bir_lowering=False,
        debug=False,
        num_devices=NCORES,
    )

    d_xT = nc.dram_tensor("xT", [D, B], F16, kind="ExternalInput").ap()
    d_attn_w = nc.dram_tensor("attn_w", [D, P], F16, kind="ExternalInput").ap()
    d_attn_b = nc.dram_tensor("attn_b", [P, 1], F32, kind="ExternalInput").ap()
    d_m3x = nc.dram_tensor("m3x", [P, D + 1], F16, kind="ExternalInput").ap()
    d_m2 = nc.dram_tensor("m2", [P, VS], F16, kind="ExternalInput").ap()
    d_out = nc.dram_tensor("out", [B, VS], F16, kind="ExternalOutput").ap()

    with tile.TileContext(nc) as tc:
        with (
            tc.tile_pool(name="consts", bufs=1) as cpool,
            tc.tile_pool(name="grp", bufs=3) as gpool,
            tc.tile_pool(name="small", bufs=3) as spool,
            tc.tile_pool(name="stage", bufs=4) as stpool,
            tc.tile_pool(name="pso", bufs=4, space="PSUM") as pso,
        ):
            # ---- resident constants (small first so block 0 starts early)
            attn_w = cpool.tile([D, P], F16)
            nc.sync.dma_start(attn_w[:], d_attn_w[:])
            attn_b = cpool.tile([P, 1], F32)
            nc.sync.dma_start(attn_b[:], d_attn_b[:])
            m3x = cpool.tile([P, D + 1], F16)
            nc.sync.dma_start(m3x[:], d_m3x[:])

            xT = cpool.tile([D, B], F16)
            m2 = cpool.tile([P, VS], F16)
            for c in range(8):  # chunked so batch tile 0 can start early
                nc.sync.dma_start(
                    xT[:, c * (B // 8):(c + 1) * (B // 8)],
                    d_xT[:, c * (B // 8):(c + 1) * (B // 8)],
                )
                if c == 0:
                    nc.sync.dma_start(m2[:], d_m2[:])

            # HAM warm-up: ~4-5us of back-to-back matmuls flips the PE
            # clock gate to 8/8 (2.4 GHz); the steady state afterwards has
            # no >=3.4us PE-idle window, so the PE stays warm for the whole
            # kernel.  One accumulation group, one stationary load, one
            # reader so nothing dangles.
            ps_wu = pso.tile([P, W], F32, tag="o", name="ps_wu")
            NWU = 16
            for wu in range(NWU):
                nc.tensor.matmul(
                    ps_wu[:], attn_w[:], xT[:, 0:W],
                    start=(wu == 0), stop=(wu == NWU - 1),
                )
            wu_junk = spool.tile([P, W], F16, tag="wuj", name="wu_junk")
            nc.vector.tensor_copy(wu_junk[:], ps_wu[:])

            # unnormalized softmax numerators, transposed
            ewT = cpool.tile([P, B], F16)

            scls = {}
            drain_st = [0.0]

            def phase_a(g):
                # logits + exp, W-wide blocks (PSUM slots borrowed from pso)
                for blk in range(G * BT // W):
                    c0 = (g * G) * BT + blk * W
                    ps_lg = pso.tile([P, W], F32, tag="o", name=f"ps_lg_{c0}")
                    nc.tensor.matmul(
                        ps_lg[:], attn_w[:], xT[:, c0:c0 + W],
                        start=True, stop=True,
                    )
                    nc.scalar.activation(
                        ewT[:, c0:c0 + W], ps_lg[:], AF.Exp, bias=attn_b[:]
                    )

                dall = gpool.tile([BT, G], F32, tag="dall", name=f"dall_{g}")
                qall = gpool.tile([BT, G], F32, tag="qall", name=f"qall_{g}")

                # parScaled (self_b folded into m3x rows) plus the
                # denominator column (m3x's trailing ones column), K=64
                for tg in range(G):
                    i = g * G + tg
                    c = i * BT
                    ps_pd = pso.tile([BT, D + 1], F32, tag="o", name=f"ps_pd_{i}")
                    nc.tensor.matmul(
                        ps_pd[:], ewT[:, c:c + BT], m3x[:],
                        start=True, stop=True,
                    )
                    # q = ||parScaled||^2 on DVE: fp16 copy out of PSUM,
                    # 2x fp16 square, tensor_scalar accumulate
                    pdsb = spool.tile([BT, D], F16, tag="pdsb", name=f"pdsb_{i}")
                    nc.vector.tensor_copy(pdsb[:], ps_pd[:, 0:D])
                    sq = spool.tile([BT, D], F16, tag="sq", name=f"sq_{i}")
                    nc.vector.tensor_mul(sq[:], pdsb[:], pdsb[:])
                    sqj = spool.tile([BT, D], F16, tag="sqj", name=f"sqj_{i}")
                    nc.vector.tensor_scalar(
                        sqj[:], sq[:], 1.0, 0.0, mybir.AluOpType.mult,
                        mybir.AluOpType.add, accum_out=qall[:, tg:tg + 1],
                    )
                    nc.vector.tensor_copy(dall[:, tg:tg + 1], ps_pd[:, D:D + 1])

                # gate: scl = sigmoid(sqrt(q)/d)/d with Ln/Exp only
                rd = gpool.tile([BT, G], F32, tag="rd", name=f"rd_{g}")
                nc.vector.reciprocal(rd[:], dall[:])
                lnq = gpool.tile([BT, G], F32, tag="lnq", name=f"lnq_{g}")
                nc.scalar.activation(lnq[:], qall[:], AF.Ln)
                smag = gpool.tile([BT, G], F32, tag="smag", name=f"smag_{g}")
                nc.scalar.activation(smag[:], lnq[:], AF.Exp, scale=0.5)
                mag = gpool.tile([BT, G], F32, tag="mag", name=f"mag_{g}")
                nc.vector.tensor_mul(mag[:], smag[:], rd[:])
                emn = gpool.tile([BT, G], F32, tag="emn", name=f"emn_{g}")
                nc.scalar.activation(emn[:], mag[:], AF.Exp, scale=-1.0)
                sp1 = gpool.tile([BT, G], F32, tag="sp1", name=f"sp1_{g}")
                nc.vector.tensor_scalar_add(sp1[:], emn[:], 1.0)
                sig = gpool.tile([BT, G], F32, tag="sig", name=f"sig_{g}")
                nc.vector.reciprocal(sig[:], sp1[:])
                scl = gpool.tile([BT, G], F32, tag="scl", name=f"scl_{g}")
                nc.vector.tensor_mul(scl[:], sig[:], rd[:])
                scls[g] = scl

            def phase_b(g):
                # projection, scale folded into the PSUM drain
                scl = scls.pop(g)
                for tg in range(G):
                    i = g * G + tg
                    c = i * BT
                    sc = scl[:, tg:tg + 1]
                    ob = stpool.tile([BT, VS], F16, tag="ob", name=f"ob_{i}")
                    for jv in range(VS // PCH + 1):
                        w = min(PCH, VS - jv * PCH)
                        ps2 = pso.tile([BT, PCH], F32, tag="o", name=f"ps2_{i}_{jv}")
                        h0 = 0
                        while h0 < w:
                            hw = min(512, w - h0)
                            off = jv * PCH + h0
                            nc.tensor.matmul(
                                ps2[:, h0:h0 + hw],
                                ewT[0:P, c:c + BT], m2[:, off:off + hw],
                                start=True, stop=True,
                            )
                            h0 += hw
                        # split drains across ACT/DVE per measured rates
                        # (ACT ~1.15us, DVE ~1.24us per [128,1024] chunk)
                        dst = ob[:, jv * PCH:jv * PCH + w]
                        drain_st[0] += ACT_FRAC
                        if drain_st[0] >= 1.0:
                            drain_st[0] -= 1.0
                            nc.scalar.activation(
                                dst, ps2[:, 0:w], AF.Identity, scale=sc
                            )
                        else:
                            nc.vector.tensor_scalar_mul(dst, ps2[:, 0:w], sc)
                    nc.sync.dma_start(d_out[i * BT:(i + 1) * BT, :], ob[:])

            # software pipeline: phase A runs one group ahead of phase B
            # so the gate latency hides under the previous group's drains
            NG = NBT // G
            phase_a(0)
            for g in range(NG):
                if g + 1 < NG:
                    phase_a(g + 1)
                phase_b(g)

    nc.compile()
    return nc


def _get_nc():
    if "nc" not in _cache:
        _cache["nc"] = _build()
    return _cache["nc"]


def make_in_maps(x, pattern_dict, attn_w, attn_b, self_w, self_b, out_w, out_b):
    x = np.asarray(x, dtype=np.float32)
    pattern_dict = np.asarray(pattern_dict, dtype=np.float32)
    attn_w = np.asarray(attn_w, dtype=np.float32)
    attn_b = np.asarray(attn_b, dtype=np.float32)
    self_w = np.asarray(self_w, dtype=np.float32)
    self_b = np.asarray(self_b, dtype=np.float32)
    out_w = np.asarray(out_w, dtype=np.float32)
    out_b = np.asarray(out_b, dtype=np.float32)

    # self_b folds into every row of m1: with unnormalized weights ewT,
    # sum_p ewT[p,b]*(m1[p,:] + self_b) = denom[b]*par[b,:] exactly.
    m3x = np.zeros((P, D + 1), dtype=np.float32)
    m3x[:, 0:D] = pattern_dict @ (self_w - np.eye(D, dtype=np.float32)) + self_b
    m3x[:, D] = 1.0
    m2 = pattern_dict @ out_w  # [P, V]

    shared = {
        "xT": np.ascontiguousarray(x.T).astype(np.float16),
        "attn_w": attn_w.astype(np.float16),
        "attn_b": np.ascontiguousarray(attn_b.reshape(P, 1)),
        "m3x": m3x.astype(np.float16),
    }
    in_maps = []
    for c in range(NCORES):
        m = dict(shared)
        m["m2"] = np.ascontiguousarray(m2[:, c * VS:(c + 1) * VS]).astype(np.float16)
        in_maps.append(m)
    return in_maps


def kernel(x, pattern_dict, attn_w, attn_b, self_w, self_b, out_w, out_b):
    in_maps = make_in_maps(
        x, pattern_dict, attn_w, attn_b, self_w, self_b, out_w, out_b
    )
    nc = _get_nc()
    res = run_bass_kernel_spmd(nc, in_maps, list(range(NCORES)))
    out = np.concatenate(
        [res.results[c]["out"].astype(np.float32) for c in range(NCORES)], axis=1
    )
    out_b = np.asarray(out_b, dtype=np.float32)
    if np.any(out_b):
        out += out_b
    return out
